# revision 3
# baseline (speedup 1.0000x reference)
"""GCN message-passing + FFN kernel for Trainium2 (8 NeuronCores).

Strategy (dst-sharded, zero collectives):
  - Sort edges by dst on host (index-only prep), pad nodes to 50176 = 8*49*128.
  - Core c owns dst rows [c*6272, (c+1)*6272): it processes every edge whose
    dst lands in its range, so partial aggregates never cross cores.
  - Per dst-block of 128 nodes: gather x[src] rows with dma_gather (Q7
    CounterMachine, one op per table half — int16 indices force a lo/hi table
    split at 32768), build a scaled one-hot mask [edge, dst_local] =
    coef[e] * (dstl[e]==q) with one fused DVE tensor_scalar(is_equal, mult),
    and matmul-accumulate aggT[feat, dst] in PSUM over the block's chunks.
  - Per-block chunk counts are the max over the 8 cores for that block slot
    (SPMD: one program, per-core data), minimizing padding.
  - Edge coefficient coef[e] = ew[e] / sqrt(deg[src]*deg[dst]) is computed on
    device in a flat vector prepass from host-gathered integer degree tables.
  - FFN fused per block: hT = relu(W1.T @ aggT + b1), out = hT.T @ W2 (+b2),
    direct DMA of the [128,128] output block to HBM.
"""
import sys

sys.path.insert(0, "/opt/trn_rl_repo")

import numpy as np

import concourse.bass as bass
import concourse.bacc as bacc
import concourse.mybir as mybir
import concourse.tile as tile
from concourse.bass_utils import run_bass_kernel_spmd

P = 128
D = 128
NCORES = 8
N_NODES = 50000
NPAD = 50176          # next multiple of 128*8 above 50000
NBLK = NPAD // P      # 392 blocks
NBC = NBLK // NCORES  # 49 blocks per core
HALF = 32768          # int16 index limit for dma_gather

f32 = mybir.dt.float32
i16 = mybir.dt.int16


def _host_pack(x, src, dst, edge_weights):
    """Index-only host prep: degree tables, dst-sort, lo/hi split by src,
    per-core columnar edge metadata + wrapped int16 gather indices."""
    E = src.shape[0]
    src = np.asarray(src).astype(np.int64)
    dst = np.asarray(dst).astype(np.int64)
    ew = np.asarray(edge_weights).astype(np.float32)

    deg = np.bincount(src, minlength=NPAD)
    deg = np.maximum(deg, 1).astype(np.float32)  # exact ints, <=2^24

    order = np.argsort(dst, kind="stable")
    ds = dst[order]
    ss = src[order]
    es = ew[order]

    g = ds >> 7                                   # global block id (dst-sorted)
    ishi = (ss >= HALF).astype(np.int64)
    key = g * 2 + ishi                            # lo edges first within block
    order2 = np.argsort(key, kind="stable")
    ds, ss, es, g, ishi, key = (a[order2] for a in (ds, ss, es, g, ishi, key))

    kcounts = np.bincount(key, minlength=NBLK * 2)
    lo_cnt = kcounts[0::2].reshape(NCORES, NBC)   # [core, slot]
    hi_cnt = kcounts[1::2].reshape(NCORES, NBC)
    # per-slot chunk counts = max over cores (SPMD single program)
    C_lo_s = np.maximum(1, np.ceil(lo_cnt.max(axis=0) / P).astype(int))
    C_hi_s = np.maximum(1, np.ceil(hi_cnt.max(axis=0) / P).astype(int))
    C_s = C_lo_s + C_hi_s
    col_off = np.concatenate([[0], np.cumsum(C_s)[:-1]])      # chunk col base
    M = int(C_s.sum())

    kstarts = np.concatenate([[0], np.cumsum(kcounts)[:-1]])
    rank = np.arange(E) - kstarts[key]            # rank within (block, lo/hi)
    b_loc = g % NBC
    slot = rank + ishi * (C_lo_s[b_loc] * P)      # slot within block
    p_lane = slot % P
    t_chunk = slot // P
    core = g // NBC
    col = col_off[b_loc] + t_chunk

    dstl_all = np.zeros((NCORES, P, M), np.float32)
    ew_all = np.zeros((NCORES, P, M), np.float32)
    degs_all = np.ones((NCORES, P, M), np.float32)
    degd_all = np.ones((NCORES, P, M), np.float32)

    dstl_all[core, p_lane, col] = (ds & 127).astype(np.float32)
    ew_all[core, p_lane, col] = es
    degs_all[core, p_lane, col] = deg[ss]
    degd_all[core, p_lane, col] = deg[ds]

    # wrapped int16 gather index arrays: per (core, block-slot), gather slot i
    # lives at [i % 16, s_off + i//16]; replicated across the 8 Q7 groups.
    def build_idx(nchunk_s, sel, values, slots):
        s_off16 = np.concatenate([[0], np.cumsum(nchunk_s * 8)[:-1]])
        ST = int((nchunk_s * 8).sum())            # int16 cols total
        arr = np.zeros((NCORES, 16, ST), np.int16)
        c, b, sl, v = core[sel], b_loc[sel], slots[sel], values[sel]
        arr[c, sl % 16, s_off16[b] + sl // 16] = v.astype(np.int16)
        return np.tile(arr, (1, 8, 1)), s_off16, ST

    is_lo = ishi == 0
    ilo16, lo_off16, ST_lo = build_idx(C_lo_s, is_lo, ss, slot)
    ihi16, hi_off16, ST_hi = build_idx(C_hi_s, ~is_lo, ss - HALF,
                                       slot - C_lo_s[b_loc] * P)

    xpad = np.zeros((NPAD, D), np.float32)
    xpad[:N_NODES] = np.asarray(x, dtype=np.float32)
    layout = dict(C_lo_s=C_lo_s.tolist(), C_hi_s=C_hi_s.tolist(),
                  col_off=col_off.tolist(), M=M,
                  lo_off16=lo_off16.tolist(), hi_off16=hi_off16.tolist(),
                  ST_lo=ST_lo, ST_hi=ST_hi)
    return layout, xpad, ilo16, ihi16, dstl_all, ew_all, degs_all, degd_all


def _build_program(layout, b2_nonzero, repeats=1, gat_bufs=4, mask_bufs=8):
    C_lo_s, C_hi_s = layout["C_lo_s"], layout["C_hi_s"]
    col_off, M = layout["col_off"], layout["M"]
    lo_off16, hi_off16 = layout["lo_off16"], layout["hi_off16"]
    ST_lo, ST_hi = layout["ST_lo"], layout["ST_hi"]
    nc = bacc.Bacc("TRN2", target_bir_lowering=False, debug=False)

    xt = nc.dram_tensor("xt", [NPAD, D], f32, kind="ExternalInput")
    ilo_d = nc.dram_tensor("ilo", [P, ST_lo], i16, kind="ExternalInput")
    ihi_d = nc.dram_tensor("ihi", [P, ST_hi], i16, kind="ExternalInput")
    dstl_d = nc.dram_tensor("dstl", [P, M], f32, kind="ExternalInput")
    ew_d = nc.dram_tensor("ew", [P, M], f32, kind="ExternalInput")
    degs_d = nc.dram_tensor("degs", [P, M], f32, kind="ExternalInput")
    degd_d = nc.dram_tensor("degd", [P, M], f32, kind="ExternalInput")
    iota_d = nc.dram_tensor("iota", [P, P], f32, kind="ExternalInput")
    w1_d = nc.dram_tensor("w1", [D, D], f32, kind="ExternalInput")
    w2_d = nc.dram_tensor("w2", [D, D], f32, kind="ExternalInput")
    b1_d = nc.dram_tensor("b1", [D, 1], f32, kind="ExternalInput")
    if b2_nonzero:
        b2b_d = nc.dram_tensor("b2b", [P, D], f32, kind="ExternalInput")
    out_d = nc.dram_tensor("out", [NBC * P, D], f32, kind="ExternalOutput")

    with tile.TileContext(nc) as tc:
        with tc.tile_pool(name="meta", bufs=1) as meta, \
             tc.tile_pool(name="gat", bufs=gat_bufs) as gat, \
             tc.tile_pool(name="msk", bufs=mask_bufs) as msk, \
             tc.tile_pool(name="eptp", bufs=3) as eptp, \
             tc.tile_pool(name="ps_agg", bufs=2, space="PSUM") as ps_agg, \
             tc.tile_pool(name="ps_h", bufs=2, space="PSUM") as ps_h, \
             tc.tile_pool(name="ps_o", bufs=2, space="PSUM") as ps_o:

            ilo_sb = meta.tile([P, ST_lo], i16)
            nc.sync.dma_start(out=ilo_sb[:], in_=ilo_d.ap())
            ihi_sb = meta.tile([P, ST_hi], i16)
            nc.sync.dma_start(out=ihi_sb[:], in_=ihi_d.ap())
            dstl_sb = meta.tile([P, M], f32)
            nc.sync.dma_start(out=dstl_sb[:], in_=dstl_d.ap())
            ew_sb = meta.tile([P, M], f32)
            nc.sync.dma_start(out=ew_sb[:], in_=ew_d.ap())
            degs_sb = meta.tile([P, M], f32)
            nc.sync.dma_start(out=degs_sb[:], in_=degs_d.ap())
            degd_sb = meta.tile([P, M], f32)
            nc.sync.dma_start(out=degd_sb[:], in_=degd_d.ap())
            iota_sb = meta.tile([P, P], f32)
            nc.sync.dma_start(out=iota_sb[:], in_=iota_d.ap())
            w1_sb = meta.tile([D, D], f32)
            nc.sync.dma_start(out=w1_sb[:], in_=w1_d.ap())
            w2_sb = meta.tile([D, D], f32)
            nc.sync.dma_start(out=w2_sb[:], in_=w2_d.ap())
            b1_sb = meta.tile([D, 1], f32)
            nc.sync.dma_start(out=b1_sb[:], in_=b1_d.ap())
            if b2_nonzero:
                b2b_sb = meta.tile([P, D], f32)
                nc.sync.dma_start(out=b2b_sb[:], in_=b2b_d.ap())

            # prepass: se = ew / sqrt(degs*degd)
            prod_sb = meta.tile([P, M], f32)
            nc.vector.tensor_tensor(out=prod_sb[:], in0=degs_sb[:],
                                    in1=degd_sb[:], op=mybir.AluOpType.mult)
            root_sb = meta.tile([P, M], f32)
            nc.scalar.sqrt(root_sb[:], prod_sb[:])
            rr_sb = meta.tile([P, M], f32)
            nc.vector.reciprocal(rr_sb[:], root_sb[:])
            se_sb = meta.tile([P, M], f32)
            nc.vector.tensor_tensor(out=se_sb[:], in0=rr_sb[:],
                                    in1=ew_sb[:], op=mybir.AluOpType.mult)

            max_C = max(C_lo_s[b] + C_hi_s[b] for b in range(NBC))
            for _ in range(repeats):
                for b in range(NBC):
                    C_lo, C_hi = C_lo_s[b], C_hi_s[b]
                    C = C_lo + C_hi
                    S_lo, S_hi = C_lo * 8, C_hi * 8
                    xg = gat.tile([P, max_C, D], f32, tag="xg")
                    nc.gpsimd.dma_gather(
                        out_ap=xg[:, 0:C_lo, :], in_ap=xt.ap()[0:HALF, :],
                        idxs_ap=ilo_sb[:, lo_off16[b]:lo_off16[b] + S_lo],
                        num_idxs=C_lo * P, num_idxs_reg=C_lo * P,
                        elem_size=D, single_packet=False)
                    nc.gpsimd.dma_gather(
                        out_ap=xg[:, C_lo:C, :], in_ap=xt.ap()[HALF:NPAD, :],
                        idxs_ap=ihi_sb[:, hi_off16[b]:hi_off16[b] + S_hi],
                        num_idxs=C_hi * P, num_idxs_reg=C_hi * P,
                        elem_size=D, single_packet=False)
                    agg_ps = ps_agg.tile([D, P], f32, tag="agg")
                    for t in range(C):
                        c = col_off[b] + t
                        mask = msk.tile([P, P], f32, tag="mask")
                        nc.vector.tensor_scalar(
                            out=mask[:], in0=iota_sb[:],
                            scalar1=dstl_sb[:, c:c + 1],
                            scalar2=se_sb[:, c:c + 1],
                            op0=mybir.AluOpType.is_equal,
                            op1=mybir.AluOpType.mult)
                        nc.tensor.matmul(out=agg_ps[:], lhsT=xg[:, t, :],
                                         rhs=mask[:],
                                         start=(t == 0), stop=(t == C - 1))
                    aggT_sb = eptp.tile([D, P], f32, tag="aggT")
                    nc.scalar.copy(aggT_sb[:], agg_ps[:])
                    h_ps = ps_h.tile([D, P], f32, tag="h")
                    nc.tensor.matmul(out=h_ps[:], lhsT=w1_sb[:], rhs=aggT_sb[:],
                                     start=True, stop=True)
                    hT_sb = eptp.tile([D, P], f32, tag="hT")
                    nc.scalar.activation(hT_sb[:], h_ps[:],
                                         mybir.ActivationFunctionType.Relu,
                                         bias=b1_sb[:, :1], scale=1.0)
                    o_ps = ps_o.tile([P, D], f32, tag="o")
                    nc.tensor.matmul(out=o_ps[:], lhsT=hT_sb[:], rhs=w2_sb[:],
                                     start=True, stop=True)
                    out_sb = eptp.tile([P, D], f32, tag="outsb")
                    if b2_nonzero:
                        nc.vector.tensor_tensor(out=out_sb[:], in0=o_ps[:],
                                                in1=b2b_sb[:],
                                                op=mybir.AluOpType.add)
                    else:
                        nc.scalar.copy(out_sb[:], o_ps[:])
                    nc.sync.dma_start(out=out_d.ap()[b * P:(b + 1) * P, :],
                                      in_=out_sb[:])
    nc.compile()
    return nc


def _make_in_maps(xpad, ilo16, ihi16, dstl_all, ew_all, degs_all, degd_all,
                  W1, b1, W2, b2, b2_nonzero):
    iota = np.tile(np.arange(P, dtype=np.float32), (P, 1))
    in_maps = []
    for c in range(NCORES):
        m = {
            "xt": xpad,
            "ilo": ilo16[c],
            "ihi": ihi16[c],
            "dstl": dstl_all[c],
            "ew": ew_all[c],
            "degs": degs_all[c],
            "degd": degd_all[c],
            "iota": iota,
            "w1": np.asarray(W1, np.float32),
            "w2": np.asarray(W2, np.float32),
            "b1": np.asarray(b1, np.float32).reshape(D, 1),
        }
        if b2_nonzero:
            m["b2b"] = np.tile(np.asarray(b2, np.float32).reshape(1, D), (P, 1))
        in_maps.append(m)
    return in_maps


def kernel(x, src, dst, edge_weights, W1, b1, W2, b2):
    layout, xpad, ilo16, ihi16, dstl_all, ew_all, degs_all, degd_all = \
        _host_pack(x, src, dst, edge_weights)
    b2_nonzero = bool(np.any(np.asarray(b2)))
    nc = _build_program(layout, b2_nonzero)
    in_maps = _make_in_maps(xpad, ilo16, ihi16, dstl_all, ew_all, degs_all,
                            degd_all, W1, b1, W2, b2, b2_nonzero)
    res = run_bass_kernel_spmd(nc, in_maps, core_ids=list(range(NCORES)))
    out = np.concatenate([res.results[c]["out"] for c in range(NCORES)], axis=0)
    return out[:N_NODES].astype(np.float32)


# revision 12
# speedup vs baseline: 2.1883x; 2.1883x over previous
"""GCN message-passing + FFN kernel for Trainium2 (8 NeuronCores).

Strategy (dst-sharded, zero collectives):
  - Sort edges by dst on host (index-only prep), pad nodes to 50176 = 8*49*128.
  - Core c owns dst rows [c*6272, (c+1)*6272): it processes every edge whose
    dst lands in its range, so partial aggregates never cross cores.
  - Per dst-block of 128 nodes: gather x[src] rows with dma_gather (Q7
    CounterMachine, one op per table half — int16 indices force a lo/hi table
    split at 32768), build a scaled one-hot mask [edge, dst_local] =
    coef[e] * (dstl[e]==q) with one fused DVE tensor_scalar(is_equal, mult),
    and matmul-accumulate aggT[feat, dst] in PSUM over the block's chunks.
  - Per-block chunk counts are the max over the 8 cores for that block slot
    (SPMD: one program, per-core data), minimizing padding.
  - Edge coefficient coef[e] = ew[e] / sqrt(deg[src]*deg[dst]) is computed on
    device in a flat vector prepass from host-gathered integer degree tables.
  - FFN fused per block: hT = relu(W1.T @ aggT + b1), out = hT.T @ W2 (+b2),
    direct DMA of the [128,128] output block to HBM.
"""
import sys

sys.path.insert(0, "/opt/trn_rl_repo")

import numpy as np

import concourse.bass as bass
import concourse.bacc as bacc
import concourse.mybir as mybir
import concourse.tile as tile
from concourse.bass_utils import run_bass_kernel_spmd

P = 128
D = 128
NCORES = 8
N_NODES = 50000
NPAD = 50176          # next multiple of 128*8 above 50000
NBLK = NPAD // P      # 392 blocks
NBC = NBLK // NCORES  # 49 blocks per core
HALF = 32768          # int16 index limit for dma_gather

f32 = mybir.dt.float32
i16 = mybir.dt.int16


def _host_pack(x, src, dst, edge_weights):
    """Index-only host prep: degree tables, dst-sort, lo/hi split by src,
    per-core columnar edge metadata + wrapped int16 gather indices."""
    E = src.shape[0]
    src = np.asarray(src).astype(np.int64)
    dst = np.asarray(dst).astype(np.int64)
    ew = np.asarray(edge_weights).astype(np.float32)

    deg = np.bincount(src, minlength=NPAD)
    deg = np.maximum(deg, 1).astype(np.float32)  # exact ints, <=2^24

    order = np.argsort(dst, kind="stable")
    ds = dst[order]
    ss = src[order]
    es = ew[order]

    g = ds >> 7                                   # global block id (dst-sorted)
    ishi = (ss >= HALF).astype(np.int64)
    key = g * 2 + ishi                            # lo edges first within block
    order2 = np.argsort(key, kind="stable")
    ds, ss, es, g, ishi, key = (a[order2] for a in (ds, ss, es, g, ishi, key))

    kcounts = np.bincount(key, minlength=NBLK * 2)
    lo_cnt = kcounts[0::2].reshape(NCORES, NBC)   # [core, slot]
    hi_cnt = kcounts[1::2].reshape(NCORES, NBC)
    # per-slot chunk counts = max over cores (SPMD single program)
    C_lo_s = np.maximum(1, np.ceil(lo_cnt.max(axis=0) / P).astype(int))
    C_hi_s = np.maximum(1, np.ceil(hi_cnt.max(axis=0) / P).astype(int))
    C_s = C_lo_s + C_hi_s
    col_off = np.concatenate([[0], np.cumsum(C_s)[:-1]])      # chunk col base
    M = int(C_s.sum())

    kstarts = np.concatenate([[0], np.cumsum(kcounts)[:-1]])
    rank = np.arange(E) - kstarts[key]            # rank within (block, lo/hi)
    b_loc = g % NBC
    slot = rank + ishi * (C_lo_s[b_loc] * P)      # slot within block
    p_lane = slot % P
    t_chunk = slot // P
    core = g // NBC
    col = col_off[b_loc] + t_chunk

    dstl_all = np.zeros((NCORES, P, M), np.float32)
    ew_all = np.zeros((NCORES, P, M), np.float32)
    degs_all = np.ones((NCORES, P, M), np.float32)
    degd_all = np.ones((NCORES, P, M), np.float32)

    dstl_all[core, p_lane, col] = (ds & 127).astype(np.float32)
    ew_all[core, p_lane, col] = es
    degs_all[core, p_lane, col] = deg[ss]
    degd_all[core, p_lane, col] = deg[ds]

    # wrapped int16 gather index arrays: per (core, block-slot), gather slot i
    # lives at [i % 16, s_off + i//16]; replicated across the 8 Q7 groups.
    def build_idx(nchunk_s, sel, values, slots):
        s_off16 = np.concatenate([[0], np.cumsum(nchunk_s * 8)[:-1]])
        ST = int((nchunk_s * 8).sum())            # int16 cols total
        arr = np.zeros((NCORES, 16, ST), np.int16)
        c, b, sl, v = core[sel], b_loc[sel], slots[sel], values[sel]
        arr[c, sl % 16, s_off16[b] + sl // 16] = v.astype(np.int16)
        return np.tile(arr, (1, 8, 1)), s_off16, ST

    is_lo = ishi == 0
    ilo16, lo_off16, ST_lo = build_idx(C_lo_s, is_lo, ss, slot)
    ihi16, hi_off16, ST_hi = build_idx(C_hi_s, ~is_lo, ss - HALF,
                                       slot - C_lo_s[b_loc] * P)

    xpad = np.zeros((NPAD, D), np.float32)
    xpad[:N_NODES] = np.asarray(x, dtype=np.float32)
    layout = dict(C_lo_s=C_lo_s.tolist(), C_hi_s=C_hi_s.tolist(),
                  col_off=col_off.tolist(), M=M,
                  lo_off16=lo_off16.tolist(), hi_off16=hi_off16.tolist(),
                  ST_lo=ST_lo, ST_hi=ST_hi)
    return layout, xpad, ilo16, ihi16, dstl_all, ew_all, degs_all, degd_all


def _build_program(layout, b2_nonzero, repeats=1, gat_bufs=4, mask_bufs=8):
    C_lo_s, C_hi_s = layout["C_lo_s"], layout["C_hi_s"]
    col_off, M = layout["col_off"], layout["M"]
    lo_off16, hi_off16 = layout["lo_off16"], layout["hi_off16"]
    ST_lo, ST_hi = layout["ST_lo"], layout["ST_hi"]
    nc = bacc.Bacc("TRN2", target_bir_lowering=False, debug=False,
                   num_swdge_queues=3)

    xt = nc.dram_tensor("xt", [NPAD, D], f32, kind="ExternalInput")
    ilo_d = nc.dram_tensor("ilo", [P, ST_lo], i16, kind="ExternalInput")
    ihi_d = nc.dram_tensor("ihi", [P, ST_hi], i16, kind="ExternalInput")
    dstl_d = nc.dram_tensor("dstl", [P, M], f32, kind="ExternalInput")
    ew_d = nc.dram_tensor("ew", [P, M], f32, kind="ExternalInput")
    degs_d = nc.dram_tensor("degs", [P, M], f32, kind="ExternalInput")
    degd_d = nc.dram_tensor("degd", [P, M], f32, kind="ExternalInput")
    iota_d = nc.dram_tensor("iota", [P, P], f32, kind="ExternalInput")
    w1_d = nc.dram_tensor("w1", [D, D], f32, kind="ExternalInput")
    w2_d = nc.dram_tensor("w2", [D, D], f32, kind="ExternalInput")
    b1_d = nc.dram_tensor("b1", [D, 1], f32, kind="ExternalInput")
    if b2_nonzero:
        b2b_d = nc.dram_tensor("b2b", [P, D], f32, kind="ExternalInput")
    out_d = nc.dram_tensor("out", [NBC * P, D], f32, kind="ExternalOutput")

    with tile.TileContext(nc) as tc:
        with tc.tile_pool(name="meta", bufs=1) as meta, \
             tc.tile_pool(name="gat", bufs=gat_bufs) as gat, \
             tc.tile_pool(name="msk", bufs=mask_bufs) as msk, \
             tc.tile_pool(name="eptp", bufs=3) as eptp, \
             tc.tile_pool(name="ps_agg", bufs=2, space="PSUM") as ps_agg, \
             tc.tile_pool(name="ps_h", bufs=2, space="PSUM") as ps_h, \
             tc.tile_pool(name="ps_o", bufs=2, space="PSUM") as ps_o, \
             tc.tile_pool(name="ps_c", bufs=1, space="PSUM") as ps_c:

            ilo_sb = meta.tile([P, ST_lo], i16)
            nc.sync.dma_start(out=ilo_sb[:], in_=ilo_d.ap())
            ihi_sb = meta.tile([P, ST_hi], i16)
            nc.sync.dma_start(out=ihi_sb[:], in_=ihi_d.ap())
            dstl_sb = meta.tile([P, M], f32)
            nc.sync.dma_start(out=dstl_sb[:], in_=dstl_d.ap())
            ew_sb = meta.tile([P, M], f32)
            nc.sync.dma_start(out=ew_sb[:], in_=ew_d.ap())
            degs_sb = meta.tile([P, M], f32)
            nc.sync.dma_start(out=degs_sb[:], in_=degs_d.ap())
            degd_sb = meta.tile([P, M], f32)
            nc.sync.dma_start(out=degd_sb[:], in_=degd_d.ap())
            iota_sb = meta.tile([P, P], f32)
            nc.sync.dma_start(out=iota_sb[:], in_=iota_d.ap())
            # iota kept in PSUM: the mask tensor_scalar reads it via the
            # PSUM port, keeping the DVE entirely off the SBUF ports that
            # the gather writes + GPSIMD SWDGE descriptor generation need.
            # (Measured: any SBUF-sourced mask variant stalls the gathers.)
            iota_ps = ps_c.tile([P, P], f32)
            nc.scalar.copy(iota_ps[:], iota_sb[:])
            w1_sb = meta.tile([D, D], f32)
            nc.sync.dma_start(out=w1_sb[:], in_=w1_d.ap())
            w2_sb = meta.tile([D, D], f32)
            nc.sync.dma_start(out=w2_sb[:], in_=w2_d.ap())
            b1_sb = meta.tile([D, 1], f32)
            nc.sync.dma_start(out=b1_sb[:], in_=b1_d.ap())
            if b2_nonzero:
                b2b_sb = meta.tile([P, D], f32)
                nc.sync.dma_start(out=b2b_sb[:], in_=b2b_d.ap())

            # prepass: se = ew / sqrt(degs*degd)
            prod_sb = meta.tile([P, M], f32)
            nc.vector.tensor_tensor(out=prod_sb[:], in0=degs_sb[:],
                                    in1=degd_sb[:], op=mybir.AluOpType.mult)
            root_sb = meta.tile([P, M], f32)
            nc.scalar.sqrt(root_sb[:], prod_sb[:])
            rr_sb = meta.tile([P, M], f32)
            nc.vector.reciprocal(rr_sb[:], root_sb[:])
            se_sb = meta.tile([P, M], f32)
            nc.vector.tensor_tensor(out=se_sb[:], in0=rr_sb[:],
                                    in1=ew_sb[:], op=mybir.AluOpType.mult)
            ndstl_sb = meta.tile([P, M], f32)
            nc.vector.tensor_scalar(out=ndstl_sb[:], in0=dstl_sb[:],
                                    scalar1=-1.0, scalar2=None,
                                    op0=mybir.AluOpType.mult)

            max_C = max(C_lo_s[b] + C_hi_s[b] for b in range(NBC))
            for _ in range(repeats):
                for b in range(NBC):
                    C_lo, C_hi = C_lo_s[b], C_hi_s[b]
                    C = C_lo + C_hi
                    S_lo, S_hi = C_lo * 8, C_hi * 8
                    xg = gat.tile([P, max_C, D], f32, tag="xg")
                    nc.gpsimd.dma_gather(
                        out_ap=xg[:, 0:C_lo, :], in_ap=xt.ap()[0:HALF, :],
                        idxs_ap=ilo_sb[:, lo_off16[b]:lo_off16[b] + S_lo],
                        num_idxs=C_lo * P, num_idxs_reg=C_lo * P,
                        elem_size=D, single_packet=False,
                        queue_num=(2 * b) % 3)
                    nc.gpsimd.dma_gather(
                        out_ap=xg[:, C_lo:C, :], in_ap=xt.ap()[HALF:NPAD, :],
                        idxs_ap=ihi_sb[:, hi_off16[b]:hi_off16[b] + S_hi],
                        num_idxs=C_hi * P, num_idxs_reg=C_hi * P,
                        elem_size=D, single_packet=False,
                        queue_num=(2 * b + 1) % 3)
                    agg_ps = ps_agg.tile([D, P], f32, tag="agg")
                    for t in range(C):
                        c = col_off[b] + t
                        mask = msk.tile([P, P], f32, tag="mask")
                        if t % 3 == 2:
                            # ACT-built mask: se * relu(1 - |iota - dstl|)
                            # (exact for integer-valued iota/dstl). Offloads
                            # the DVE, and ACT's SBUF ports are private so
                            # the gathers are unaffected.
                            ad = msk.tile([P, P], f32, tag="actm")
                            nc.scalar.activation(
                                ad[:], iota_sb[:],
                                mybir.ActivationFunctionType.Abs,
                                bias=ndstl_sb[:, c:c + 1], scale=1.0)
                            rl = msk.tile([P, P], f32, tag="actr")
                            nc.scalar.activation(
                                rl[:], ad[:],
                                mybir.ActivationFunctionType.Relu,
                                bias=1.0, scale=-1.0)
                            nc.scalar.activation(
                                mask[:], rl[:],
                                mybir.ActivationFunctionType.Identity,
                                bias=0.0, scale=se_sb[:, c:c + 1])
                        else:
                            nc.vector.tensor_scalar(
                                out=mask[:], in0=iota_ps[:],
                                scalar1=dstl_sb[:, c:c + 1],
                                scalar2=se_sb[:, c:c + 1],
                                op0=mybir.AluOpType.is_equal,
                                op1=mybir.AluOpType.mult)
                        nc.tensor.matmul(out=agg_ps[:], lhsT=xg[:, t, :],
                                         rhs=mask[:],
                                         start=(t == 0), stop=(t == C - 1))
                    aggT_sb = eptp.tile([D, P], f32, tag="aggT")
                    nc.scalar.copy(aggT_sb[:], agg_ps[:])
                    h_ps = ps_h.tile([D, P], f32, tag="h")
                    nc.tensor.matmul(out=h_ps[:], lhsT=w1_sb[:], rhs=aggT_sb[:],
                                     start=True, stop=True)
                    hT_sb = eptp.tile([D, P], f32, tag="hT")
                    nc.scalar.activation(hT_sb[:], h_ps[:],
                                         mybir.ActivationFunctionType.Relu,
                                         bias=b1_sb[:, :1], scale=1.0)
                    o_ps = ps_o.tile([P, D], f32, tag="o")
                    nc.tensor.matmul(out=o_ps[:], lhsT=hT_sb[:], rhs=w2_sb[:],
                                     start=True, stop=True)
                    out_sb = eptp.tile([P, D], f32, tag="outsb")
                    if b2_nonzero:
                        nc.vector.tensor_tensor(out=out_sb[:], in0=o_ps[:],
                                                in1=b2b_sb[:],
                                                op=mybir.AluOpType.add)
                    else:
                        nc.scalar.copy(out_sb[:], o_ps[:])
                    nc.sync.dma_start(out=out_d.ap()[b * P:(b + 1) * P, :],
                                      in_=out_sb[:])
    nc.compile()
    return nc


def _make_in_maps(xpad, ilo16, ihi16, dstl_all, ew_all, degs_all, degd_all,
                  W1, b1, W2, b2, b2_nonzero):
    iota = np.tile(np.arange(P, dtype=np.float32), (P, 1))
    in_maps = []
    for c in range(NCORES):
        m = {
            "xt": xpad,
            "ilo": ilo16[c],
            "ihi": ihi16[c],
            "dstl": dstl_all[c],
            "ew": ew_all[c],
            "degs": degs_all[c],
            "degd": degd_all[c],
            "iota": iota,
            "w1": np.asarray(W1, np.float32),
            "w2": np.asarray(W2, np.float32),
            "b1": np.asarray(b1, np.float32).reshape(D, 1),
        }
        if b2_nonzero:
            m["b2b"] = np.tile(np.asarray(b2, np.float32).reshape(1, D), (P, 1))
        in_maps.append(m)
    return in_maps


def kernel(x, src, dst, edge_weights, W1, b1, W2, b2):
    layout, xpad, ilo16, ihi16, dstl_all, ew_all, degs_all, degd_all = \
        _host_pack(x, src, dst, edge_weights)
    b2_nonzero = bool(np.any(np.asarray(b2)))
    nc = _build_program(layout, b2_nonzero)
    in_maps = _make_in_maps(xpad, ilo16, ihi16, dstl_all, ew_all, degs_all,
                            degd_all, W1, b1, W2, b2, b2_nonzero)
    res = run_bass_kernel_spmd(nc, in_maps, core_ids=list(range(NCORES)))
    out = np.concatenate([res.results[c]["out"] for c in range(NCORES)], axis=0)
    return out[:N_NODES].astype(np.float32)
